# revision 51
# baseline (speedup 1.0000x reference)
"""Trainium2 Bass kernel for nn_AverageAttention: cumulative-average attention
with a sigmoid gating Linear(2D->2D).

Strategy: data-parallel over batch (B=8 = one batch element per NeuronCore).
All on-chip work happens in transposed space ([feature, token]).
  - cumavg via the affine recurrence avg_t = coef_t*avg_{t-1} + x_t/(t+1),
    one fused tensor_tensor_scan per 512-col chunk on VectorE (host
    pre-scales xdiv = x/(t+1)); chunks chained through a carry tile
  - gating matmul in fp8-e4m3 with perf_mode=DoubleRow: each instruction
    contracts TWO 128-deep k-tiles at ~2x bf16 throughput (216ns/instr;
    2048 matmuls = 442us PE floor; this kernel measures ~478us).
  - engine split keeps every in-order stream free of long-wait heads:
    DVE = scans + carry + (sig_f*avg8) mul + final add + s2/s3 fp8
    casts; Scalar = sigmoids + s0/s1 fp8 casts + x8-s0/coef/bias/xep
    issues; GpSimd = xd + x8-s1..3 loads, (sig_i*x) mul, batched avgT
    stores; Sync = W stream + outT stores.
  - avgT stores are emitted as DEP-FREE batches after the next slice's
    scans (avgc bufs=26 makes the WAR legal): a scan-dependent store
    issued early head-blocks the xd loads behind it in the queue
    (~10us PE stall in the old layout). Same rule everywhere: big or
    dependent transfers never share a queue with latency-critical
    issues (x8-s1..3 ride gpsimd, not scalar, else their 1MB clogs the
    scalar DMA-sem slots and stalls the xep->sigmoid->psum chain).
  - bias is pre-transposed on host to [P, 32] so its DMA is contiguous
    (was 4096 strided 4B descriptors hogging the W queue head for 8us).
  - startup is HBM-bound (~290 GB/s demanded 9-35us): runway W is
    split across sync (kp0 per-unit + kp1 per-pair + kp2-3 + wra) and
    scalar (kp4-7 interleaved between x8 pairs); coef rides gpsimd.
    First matmul fires at ~10us (64KB kp0-u0 chunk + first x8 pair).
  - runway avg-halves consume k-pair-major ACROSS units so the serial
    scan chain (1.27us/chunk) stays ahead of the PE.
  - pass-A tail is staggered (s0 of unit i, then s1 of unit i-1) to
    give the slice-1 scan chain one extra unit-slice of runway.
  - sigmoids and gating output are bf16 (outT DRAM bf16, host casts up;
    halves output traffic; 2x DVE throughput on mul/add).
  - pass 2 is i-outer with the 3-buf W pool giving multi-unit prefetch
    across the pass boundary.
"""
import sys

if "/opt/trn_rl_repo" not in sys.path:
    sys.path.insert(0, "/opt/trn_rl_repo")

import numpy as np
import ml_dtypes

B, T, D = 8, 2048, 2048
O = 2 * D          # gate output features (4096)
P = 128            # partitions
KT = D // P        # 16 k-tiles per half of G
DT = D // P        # 16 output-feature tiles
NK = 2 * KT        # 32 k-tiles total
NPAIR = NK // 2    # 16 DoubleRow k-pairs (8 x-pairs + 8 avg-pairs)
TS = 512           # t-slice (matmul moving free dim / scan chunk)
NS = T // TS       # 4 t-slices
RUNWAY = 4         # units whose x-half matmuls front-run the scans

_compiled = None


def _build():
    import concourse.mybir as mybir
    import concourse.tile as tile
    from concourse import bacc

    f32 = mybir.dt.float32
    bf16 = mybir.dt.bfloat16
    f8 = mybir.dt.float8e4
    SIG = mybir.ActivationFunctionType.Sigmoid
    COPY = mybir.ActivationFunctionType.Copy
    DR = mybir.MatmulPerfMode.DoubleRow

    nc = bacc.Bacc(trn_type="TRN2", target_bir_lowering=False, debug=False,
                   num_devices=B)

    xT_d = nc.declare_dram_parameter("xT", [D, T], bf16, isOutput=False)
    x8S_d = nc.declare_dram_parameter("x8S", [NS, P, KT * TS], f8,
                                      isOutput=False)
    xdT_d = nc.declare_dram_parameter("xdT", [D, T], bf16, isOutput=False)
    wP_d = nc.declare_dram_parameter("wP", [DT, P, NK * 2 * P], f8,
                                     isOutput=False)
    wRWx_d = nc.declare_dram_parameter("wRWx", [P, 8, RUNWAY, 2, 2 * P], f8,
                                       isOutput=False)
    wRWa_d = nc.declare_dram_parameter("wRWa", [P, 8, RUNWAY, 2, 2 * P], f8,
                                       isOutput=False)
    # bias pre-transposed on host to [P, O//P]: contiguous 128B rows
    bias_d = nc.declare_dram_parameter("bias", [P, O // P], f32,
                                       isOutput=False)
    coef_d = nc.declare_dram_parameter("coef_t", [1, T], f32, isOutput=False)
    avgT_d = nc.declare_dram_parameter("avgT", [D, T], bf16, isOutput=True)
    outT_d = nc.declare_dram_parameter("outT", [D, T], bf16, isOutput=True)

    with tile.TileContext(nc) as tc:
        with tc.tile_pool(name="consts", bufs=1) as consts, \
             tc.tile_pool(name="resid", bufs=1) as resid, \
             tc.tile_pool(name="xdp", bufs=24) as xdp, \
             tc.tile_pool(name="xep", bufs=6) as xep_pool, \
             tc.tile_pool(name="wpool", bufs=3) as wpool, \
             tc.tile_pool(name="avgc", bufs=26) as avgc, \
             tc.tile_pool(name="sigp", bufs=8) as sigp, \
             tc.tile_pool(name="mp", bufs=8) as mp, \
             tc.tile_pool(name="outp", bufs=4) as outp, \
             tc.tile_pool(name="psum", bufs=8, space="PSUM") as pp:

            x8_s = [resid.tile([P, KT, TS], f8, name=f"x8_s{s}")
                    for s in range(NS)]
            avg8_s = [resid.tile([P, KT, TS], f8, name=f"avg8_s{s}")
                      for s in range(NS)]
            # runway x-half W split into two tiles: kp0-3 stream on the
            # sync queue, kp4-7 on the scalar queue (separate tiles so
            # write-region merging can't chain the two queues)
            wrxA = resid.tile([P, 4, RUNWAY, 2, 2 * P], f8)
            wrxB = resid.tile([P, 4, RUNWAY, 2, 2 * P], f8)
            wra = resid.tile([P, 8, RUNWAY, 2, 2 * P], f8)

            x8v = x8S_d.rearrange("s p (kt c) -> s p kt c", kt=KT)
            wv = wP_d.rearrange("i p (kt c) -> i p kt c", kt=NK)

            def load_w(i):
                """Two half-tile DMAs on the SAME queue: the x-half
                (kt 0-15) lands first so its matmuls' dependency fires
                at 512KB instead of waiting for the full 1MB tile."""
                w_i = wpool.tile([P, NK, 2 * P], f8, tag="w", name="w_i")
                nc.sync.dma_start(out=w_i[:, 0:KT, :],
                                  in_=wv[i, :, 0:KT, :])
                nc.sync.dma_start(out=w_i[:, KT:NK, :],
                                  in_=wv[i, :, KT:NK, :])
                return w_i

            # startup streams, earliest consumers first. The runway W
            # demand (~150 GB/s) is split across TWO queues so neither
            # runs at its bandwidth edge:
            #   sync q:   wrx kp0 (per-unit), kp1 (per-pair), kp2, kp3,
            #             then wra q0-7
            #   scalar q: x8 pairs interleaved with wrx kp4-7, coef,
            #             bias
            #   gpsimd q: xd s0, xd s1 (issue_xd below)
            for u in range(RUNWAY):
                nc.sync.dma_start(out=wrxA[:, 0, u, :, :],
                                  in_=wRWx_d[:, 0, u, :, :])
            nc.sync.dma_start(out=wrxA[:, 1, 0:2, :, :],
                              in_=wRWx_d[:, 1, 0:2, :, :])
            nc.sync.dma_start(out=wrxA[:, 1, 2:4, :, :],
                              in_=wRWx_d[:, 1, 2:4, :, :])
            for kp in (2, 3):
                nc.sync.dma_start(out=wrxA[:, kp, :, :, :],
                                  in_=wRWx_d[:, kp, :, :, :])
            for q in range(8):
                nc.sync.dma_start(out=wra[:, q, :, :, :],
                                  in_=wRWa_d[:, q, :, :, :])

            coef_sb = consts.tile([P, T], f32)
            bias_sb = consts.tile([P, O // P], f32)
            # coef s0 rides gpsimd AHEAD of the xd issues (emitted just
            # below): scalar stays pure x8+wrxB during the runway
            nc.gpsimd.dma_start(out=coef_sb[:, 0:TS],
                                in_=coef_d[:, 0:TS].to_broadcast((P, TS)))
            nc.scalar.dma_start(out=x8_s[0][:, 0:2, :],
                                in_=x8v[0, :, 0:2, :])
            nc.scalar.dma_start(out=x8_s[0][:, 2:4, :],
                                in_=x8v[0, :, 2:4, :])
            nc.scalar.dma_start(out=x8_s[0][:, 4:6, :],
                                in_=x8v[0, :, 4:6, :])
            for kp in (4, 5, 6):
                nc.scalar.dma_start(out=wrxB[:, kp - 4, :, :, :],
                                    in_=wRWx_d[:, kp, :, :, :])
                a = (kp - 1) * 2
                nc.scalar.dma_start(out=x8_s[0][:, a:a + 2, :],
                                    in_=x8v[0, :, a:a + 2, :])
            for a in range(12, KT, 2):
                nc.scalar.dma_start(out=x8_s[0][:, a:a + 2, :],
                                    in_=x8v[0, :, a:a + 2, :])
            nc.scalar.dma_start(out=wrxB[:, 3, :, :, :],
                                in_=wRWx_d[:, 7, :, :, :])
            nc.scalar.dma_start(out=bias_sb, in_=bias_d[:, :])

            carry = consts.tile([P, KT], f32)
            xd_tiles = {}

            def issue_xd(s):
                """DMA issues for all 16 xd chunks of slice s, all on the
                gpsimd queue (which carries nothing scan-dependent, so
                the issues flow back-to-back). Putting any on the scalar
                queue throttles them behind the x8 stream via the shared
                rotating DMA-semaphore slots."""
                sl = slice(s * TS, (s + 1) * TS)
                for j in range(KT):
                    rows = slice(j * P, (j + 1) * P)
                    xd = xdp.tile([P, TS], bf16, tag="xd", name="xd")
                    nc.gpsimd.dma_start(out=xd, in_=xdT_d[rows, sl])
                    xd_tiles[(s, j)] = xd

            def load_x8(s):
                # gpsimd queue (idle mid-kernel): a 1MB x8 load on the
                # scalar queue clogs its DMA-sem slots for ~9us, holding
                # the xep-issue -> sigmoid chain and stalling psum reuse
                nc.gpsimd.dma_start(out=x8_s[s][:, 0:8, :],
                                    in_=x8v[s, :, 0:8, :])
                nc.gpsimd.dma_start(out=x8_s[s][:, 8:KT, :],
                                    in_=x8v[s, :, 8:KT, :])

            def load_coef(s):
                sl = slice(s * TS, (s + 1) * TS)
                nc.scalar.dma_start(
                    out=coef_sb[:, sl],
                    in_=coef_d[:, sl].to_broadcast((P, TS)))

            avc_tiles = {}

            def scan_set(s, jlo=0, jhi=KT, cast_eng="scalar"):
                """Scans for chunks [jlo,jhi) of slice s. xd tiles must
                already be issued via issue_xd. s0/s1 casts ride Scalar
                (prompt, keeps DVE lean early); s2/s3 casts ride DVE
                right behind their scans so they never head-block the
                Scalar sigmoid stream. avgT stores are NOT issued here:
                a scan-dependent store issued early head-blocks the xd
                loads behind it in the queue (costs ~10us); store_avg
                emits them later as a dep-free batch."""
                sl = slice(s * TS, (s + 1) * TS)
                for j in range(jlo, jhi):
                    avc = avgc.tile([P, TS], bf16, tag="avc", name="avc")
                    nc.vector.tensor_tensor_scan(
                        out=avc, data0=coef_sb[:, sl],
                        data1=xd_tiles.pop((s, j)),
                        initial=(0.0 if s == 0 else carry[:, j:j + 1]),
                        op0=mybir.AluOpType.mult, op1=mybir.AluOpType.add)
                    if s < NS - 1:
                        nc.vector.tensor_copy(carry[:, j:j + 1],
                                              avc[:, TS - 1:TS])
                    if cast_eng == "scalar":
                        nc.scalar.activation(avg8_s[s][:, j, :], avc, COPY)
                    else:
                        nc.vector.tensor_copy(avg8_s[s][:, j, :], avc)
                    avc_tiles[(s, j)] = avc

            def store_avg(s):
                """Batched avgT stores for slice s, emitted at a point
                where every scan of slice s is already complete, so the
                issues never stall the gpsimd stream."""
                sl = slice(s * TS, (s + 1) * TS)
                for j in range(KT):
                    rows = slice(j * P, (j + 1) * P)
                    nc.gpsimd.dma_start(out=avgT_d[rows, sl],
                                        in_=avc_tiles.pop((s, j)))

            def rhs_for(kp, s):
                if kp < NPAIR // 2:
                    return x8_s[s][:, 2 * kp:2 * kp + 2, :]
                jj = 2 * (kp - NPAIR // 2)
                return avg8_s[s][:, jj:jj + 2, :]

            def mm_half(ps_ig, ps_fg, w_i, s, half):
                kps = range(0, NPAIR // 2) if half == 0 \
                    else range(NPAIR // 2, NPAIR)
                for kp in kps:
                    nc.tensor.matmul(ps_ig, lhsT=w_i[:, 2 * kp:2 * kp + 2, 0:P],
                                     rhs=rhs_for(kp, s), start=(kp == 0),
                                     stop=(kp == NPAIR - 1), perf_mode=DR)
                for kp in kps:
                    nc.tensor.matmul(ps_fg,
                                     lhsT=w_i[:, 2 * kp:2 * kp + 2, P:2 * P],
                                     rhs=rhs_for(kp, s), start=(kp == 0),
                                     stop=(kp == NPAIR - 1), perf_mode=DR)

            def epilogue(ps_ig, ps_fg, i, s):
                sl = slice(s * TS, (s + 1) * TS)
                x_ep = xep_pool.tile([P, TS], bf16, tag="xe", name="x_ep")
                nc.scalar.dma_start(out=x_ep,
                                    in_=xT_d[i * P:(i + 1) * P, sl])
                sig_i = sigp.tile([P, TS], bf16, tag="sig", name="sig_i")
                nc.scalar.activation(sig_i, ps_ig, SIG,
                                     bias=bias_sb[:, i:i + 1])
                sig_f = sigp.tile([P, TS], bf16, tag="sig", name="sig_f")
                nc.scalar.activation(sig_f, ps_fg, SIG,
                                     bias=bias_sb[:, KT + i:KT + i + 1])
                m1 = mp.tile([P, TS], bf16, tag="m", name="m1")
                nc.gpsimd.tensor_mul(m1, sig_i, x_ep)
                m2 = mp.tile([P, TS], bf16, tag="m", name="m2")
                nc.vector.tensor_mul(m2, sig_f, avg8_s[s][:, i, :])
                out_s = outp.tile([P, TS], bf16, tag="out", name="out_s")
                nc.vector.tensor_add(out_s, m1, m2)
                # outT rides the sync HWDGE: its issue waits on the add,
                # and sync is the only stream with nothing downstream-
                # urgent (W prefetch has ~3 units of slack); scalar or
                # gpsimd placement head-blocks casts / slow SWDGE drain
                nc.sync.dma_start(out=outT_d[i * P:(i + 1) * P, sl],
                                  in_=out_s)

            def full_unit(w_i, i, s):
                ps_ig = pp.tile([P, TS], f32, tag="ps", name="ps_ig")
                ps_fg = pp.tile([P, TS], f32, tag="ps", name="ps_fg")
                mm_half(ps_ig, ps_fg, w_i, s, half=0)
                mm_half(ps_ig, ps_fg, w_i, s, half=1)
                epilogue(ps_ig, ps_fg, i, s)

            # ---- slice-0 and slice-1 xd pre-issued, slice-0 scans
            # ---- emitted before anything else contends for DVE
            issue_xd(0)
            issue_xd(1)
            load_x8(1)
            load_coef(1)
            scan_set(0)

            # ---- pass 1 (s = 0 across all i): runway x-halves emitted
            # ---- k-pair-major ACROSS the 4 units, then per-unit
            # ---- avg-half+epilogue; slice-1 scans spread between the
            # ---- later runway epilogues (casts must not head-block
            # ---- the RW1 sigmoid, which gates pass-A psum reuse)
            run_ps = [(pp.tile([P, TS], f32, tag="ps", name="ps_rw_i"),
                       pp.tile([P, TS], f32, tag="ps", name="ps_rw_f"))
                      for _ in range(RUNWAY)]
            for kp in range(NPAIR // 2):
                wrx_t = wrxA if kp < 4 else wrxB
                for i in range(RUNWAY):
                    ps_ig, ps_fg = run_ps[i]
                    nc.tensor.matmul(
                        ps_ig, lhsT=wrx_t[:, kp % 4, i, :, 0:P],
                        rhs=rhs_for(kp, 0), start=(kp == 0),
                        stop=False, perf_mode=DR)
                    nc.tensor.matmul(
                        ps_fg, lhsT=wrx_t[:, kp % 4, i, :, P:2 * P],
                        rhs=rhs_for(kp, 0), start=(kp == 0),
                        stop=False, perf_mode=DR)
            # avg-halves ALSO k-pair-major across units so the last
            # scan chunks (c14,c15) are needed ~10us later than with a
            # per-unit sweep — the serial scan chain stays ahead
            for q in range(8):
                for i in range(RUNWAY):
                    ps_ig, ps_fg = run_ps[i]
                    nc.tensor.matmul(
                        ps_ig, lhsT=wra[:, q, i, :, 0:P],
                        rhs=rhs_for(8 + q, 0), start=False,
                        stop=(q == 7), perf_mode=DR)
                    nc.tensor.matmul(
                        ps_fg, lhsT=wra[:, q, i, :, P:2 * P],
                        rhs=rhs_for(8 + q, 0), start=False,
                        stop=(q == 7), perf_mode=DR)
            for i in range(RUNWAY):
                ps_ig, ps_fg = run_ps[i]
                epilogue(ps_ig, ps_fg, i, 0)
                if i == 1:
                    scan_set(1, 0, 8)
                    load_coef(2)
                elif i == 2:
                    scan_set(1, 8, KT)
                elif i == 3:
                    issue_xd(2)
                    load_coef(3)

            # ---- pass A tail, staggered: s0 of unit i, then s1 of
            # ---- unit i-1 (one extra unit-slice of runway for the
            # ---- slice-1 scan chain); scans for slices 2-3 ride DVE
            # ---- with plenty of slack (needed only in pass B)
            prev = None
            for i in range(RUNWAY, DT):
                w_i = load_w(i)
                full_unit(w_i, i, 0)
                if i == 4:
                    store_avg(0)
                elif i == 6:
                    scan_set(2, 0, 8, cast_eng="vector")
                    load_x8(2)
                elif i == 7:
                    scan_set(2, 8, KT, cast_eng="vector")
                    store_avg(1)
                elif i == 9:
                    store_avg(2)
                elif i == 10:
                    issue_xd(3)
                    scan_set(3, 0, 8, cast_eng="vector")
                    load_x8(3)
                elif i == 11:
                    scan_set(3, 8, KT, cast_eng="vector")
                if prev is not None:
                    full_unit(prev[0], prev[1], 1)
                prev = (w_i, i)
            full_unit(prev[0], prev[1], 1)

            # ---- pass B: runway units catch up on slices 1-3, the
            # ---- rest on slices 2-3 (W reloaded once; the 4-buf pool
            # ---- prefetches across the pass boundary) ----
            for i in range(DT):
                w_i = load_w(i)
                for s in ((1, 2, 3) if i < RUNWAY else (2, 3)):
                    full_unit(w_i, i, s)
                if i == 0:
                    store_avg(3)

    nc.compile()
    return nc


def _get_compiled():
    global _compiled
    if _compiled is None:
        _compiled = _build()
    return _compiled


def _run(inputs, trace=False, **spmd_kwargs):
    from concourse.bass_utils import run_bass_kernel_spmd

    nc = _get_compiled()
    layer_in = np.asarray(inputs["layer_in"], dtype=np.float32)
    W_gate = np.asarray(inputs["W_gate"], dtype=np.float32)
    b_gate = np.asarray(inputs["b_gate"], dtype=np.float32)

    # wP[i, p, kt*256 + c] = W^T[kt*128 + p, gate-tile i column c]
    # (c < 128: input-gate cols i*128+c; c >= 128: forget-gate cols
    #  D + i*128 + (c-128)) — partition-contiguous 8KB rows per unit
    wT = np.ascontiguousarray(W_gate.T)                    # [k, o]
    wP = np.ascontiguousarray(
        wT.reshape(NK, P, 2, DT, P).transpose(3, 1, 0, 2, 4)
        .reshape(DT, P, NK * 2 * P)
    ).astype(ml_dtypes.float8_e4m3)
    # runway W packed in exact consumption order: x-halves k-pair-major
    # across units [p, kp, i, two, c], avg-halves unit-major [p, i, q, two, c]
    arr = np.asarray(wP[:RUNWAY]).reshape(RUNWAY, P, NK, 2 * P)
    wRWx = np.ascontiguousarray(
        arr[:, :, 0:KT, :].reshape(RUNWAY, P, 8, 2, 2 * P)
        .transpose(1, 2, 0, 3, 4))
    wRWa = np.ascontiguousarray(
        arr[:, :, KT:NK, :].reshape(RUNWAY, P, 8, 2, 2 * P)
        .transpose(1, 2, 0, 3, 4))
    tt = np.arange(T, dtype=np.float32)
    coef = (tt / (tt + 1.0)).reshape(1, T)
    inv = (1.0 / (tt + 1.0)).reshape(1, T)
    # bias_hp[p, c] = b_gate[c*128 + p]
    bias_hp = np.ascontiguousarray(
        b_gate.reshape(O // P, P).T).astype(np.float32)

    in_maps = []
    for b in range(B):
        xTb = np.ascontiguousarray(layer_in[b].T)
        # x8S[s, p, j*TS + c] = fp8(x^T[j*128 + p, s*512 + c])
        x8S = np.ascontiguousarray(
            xTb.reshape(KT, P, NS, TS).transpose(2, 1, 0, 3)
            .reshape(NS, P, KT * TS)
        ).astype(ml_dtypes.float8_e4m3)
        in_maps.append({
            "xT": xTb.astype(ml_dtypes.bfloat16),
            "x8S": x8S,
            "xdT": (xTb * inv).astype(ml_dtypes.bfloat16),
            "wP": wP,
            "wRWx": wRWx,
            "wRWa": wRWa,
            "bias": bias_hp,
            "coef_t": coef,
        })

    res = run_bass_kernel_spmd(nc, in_maps, core_ids=list(range(B)),
                               trace=trace, **spmd_kwargs)
    gating = np.empty((B, T, D), dtype=np.float32)
    avg = np.empty((B, T, D), dtype=np.float32)
    for b in range(B):
        gating[b] = res.results[b]["outT"].astype(np.float32).T
        avg[b] = res.results[b]["avgT"].astype(np.float32).T
    return (gating, avg), res


def kernel(**inputs):
    (gating, avg), _ = _run(inputs, trace=False)
    return gating, avg
